# revision 4
# baseline (speedup 1.0000x reference)
"""GQA kernel for Trainium2, 8 NeuronCores, tensor-parallel over heads.

Problem: B=1, T=2048, C=4096, 32 q-heads, 16 kv-heads, head_dim=128,
scale = 1/sqrt(32), causal. q head H uses kv head H%16.

Sharding (no collectives): core c owns q-heads {2c, 2c+1, 2c+16, 2c+17}
(local heads 0..3) and kv-heads {2c, 2c+1} (local kv 0..1). Output is a
host-side concat of per-core column slices.

Per-core design (v2 -- all bf16 matmuls, fp32 PSUM):
  x resident in SBUF as [128, half*32K + kc*1024 + t'] (16 chunk tiles,
  DMA'd in arrival-order so kc-major projection MMs start ~as the first
  chunks land).
  Projections q0..q3,k0,k1,v0,v1: strips [D=128, T] = per-t4 [128,512]
  psum accumulation over 32 kc chunks (startup: q0+k0+v0 kc-major with 6
  accumulators to track DMA arrival; later units t4-major zipped with
  attention).
  v transposed PE-wise into vt [tk,128] tiles (4 transposes per psum
  bank via the single-start trick).
  Attention (swapped-PV):
    S^T tile per (head,block,j): [128 tk, <=512 tq] = kt_j^T @ qt_slice
    P^T = exp(SCALE*S^T) (ACT) -> bf16, causal mask on diagonal tiles.
    outT[d, tq] accumulates v_j^T @ P^T_j, fd=512 (stationary v -> LDW
    hidden, unlike the fd=129 P^T-stationary variant).
    rowsum via ones-matmuls streaming P^T: j=0 full-array (0.25-valued
    [128,128] stationary, start=True covers the whole psum bank), later
    j's packed 4-at-a-time into 32-col groups (tile_position) so they
    run concurrently; bcast-reduce matmul (1/32 [128,128]) turns the
    4 partial rows into rowsum broadcast across 128 partitions;
    reciprocal (DVE) then outT * rec -> out DMA'd as [D, T] strips
    (host transposes).
  Attention blocks only need t4<=b slices, so attn(h) zips with the
  NEXT unit's projection MMs; attn3 zips with q3's descending t4 order.
"""

import numpy as np
import ml_dtypes

BF16 = ml_dtypes.bfloat16
T = 2048
C = 4096
D = 128
N_HEADS = 32
N_KV_HEADS = 16
SCALE = float(1.0 / np.sqrt(np.float32(N_HEADS)))
KC = C // 128          # 32 contraction chunks
NQH = 4                # local q heads per core
NKV = 2                # local kv heads per core
NT = T // 128          # 16 token tiles
N_CORES = 8
XCOLS = 2 * KC * 1024  # x packed cols
# mask tensor layout (cols)
MC_STRIPS = 0          # 4 strips of 512: mask_r[tk, tq] = tq >= tk + 128r
MC_IDENT = 4 * 512     # identity 128
MC_ONES32 = MC_IDENT + 128    # [128,32] ones
MC_Q128 = MC_ONES32 + 32      # [128,128] 0.25
MC_BC = MC_Q128 + 128         # [128,128] 1/32
MCOLS = MC_BC + 128

PACK_RS = True         # packed 32-col rowsum matmuls via tile_position
TRIM = True            # trim diagonal-tile free dims (b>=1)

_prog_cache = {}


def _build_program():
    if "nc" in _prog_cache:
        return _prog_cache["nc"]
    import concourse.bass as bass
    import concourse.tile as tile
    from concourse import bacc, mybir

    dt = mybir.dt
    f32 = dt.float32
    bf16 = dt.bfloat16
    EXP = mybir.ActivationFunctionType.Exp

    nc = bacc.Bacc("TRN2", target_bir_lowering=False, debug=False,
                   num_devices=N_CORES)

    xh_d = nc.dram_tensor("xh", [128, XCOLS], bf16, kind="ExternalInput").ap()
    wq_d = nc.dram_tensor("wq", [NQH, 128, C], bf16, kind="ExternalInput").ap()
    wk_d = nc.dram_tensor("wk", [NKV, 128, C], bf16, kind="ExternalInput").ap()
    wv_d = nc.dram_tensor("wv", [NKV, 128, C], bf16, kind="ExternalInput").ap()
    mask_d = nc.dram_tensor("masks", [128, MCOLS], bf16,
                            kind="ExternalInput").ap()
    out_d = nc.dram_tensor("out", [NQH * D, T], f32, kind="ExternalOutput").ap()

    with tile.TileContext(nc) as tc:
        with (
            tc.tile_pool(name="persist", bufs=1) as persist,
            tc.tile_pool(name="wpool", bufs=3) as wpool,
            tc.tile_pool(name="vtsp", bufs=1) as vtsp,
            tc.tile_pool(name="ptpool", bufs=6) as ptpool,
            tc.tile_pool(name="rsbp", bufs=1) as rsbp,
            tc.tile_pool(name="recp", bufs=2) as recp,
            tc.tile_pool(name="ofp", bufs=2) as ofp,
            tc.tile_pool(name="psA", bufs=4, space=bass.MemorySpace.PSUM) as psA,
            tc.tile_pool(name="psS", bufs=2, space=bass.MemorySpace.PSUM) as psS,
            tc.tile_pool(name="psP", bufs=2, space=bass.MemorySpace.PSUM) as psP,
        ):
            mask_sb = persist.tile([128, MCOLS], bf16, name="mask_sb",
                                   tag="mask_sb")
            nc.sync.dma_start(out=mask_sb[:], in_=mask_d[:])
            ident = mask_sb[:, MC_IDENT:MC_IDENT + 128]
            ones32 = mask_sb[:, MC_ONES32:MC_ONES32 + 32]
            q128 = mask_sb[:, MC_Q128:MC_Q128 + 128]
            bcmask = mask_sb[:, MC_BC:MC_BC + 128]

            qt = persist.tile([128, NQH * T], bf16, name="qt", tag="qt")
            kt = persist.tile([128, NKV * T], bf16, name="kt", tag="kt")
            vt = persist.tile([128, NKV * NT * 128], bf16, name="vt", tag="vt")

            xcs = [None] * 16
            wts = {}

            def dma_x(c):
                xc = persist.tile([128, 4096], bf16, name=f"xc{c}",
                                  tag=f"xc{c}")
                nc.sync.dma_start(out=xc[:], in_=xh_d[:, c * 4096:(c + 1) * 4096])
                xcs[c] = xc

            def xs(t4, kc):
                c = (t4 // 2) * 8 + kc // 4
                off = (kc % 4) * 1024 + (t4 % 2) * 512
                return xcs[c][:, off:off + 512]

            def dma_w(src, idx, key, halves=False):
                w = wpool.tile([128, C], bf16, name=f"w_{key}", tag="w")
                if halves:
                    nc.sync.dma_start(out=w[:, 0:C // 2], in_=src[idx][:, 0:C // 2])
                    wts[key] = w
                else:
                    nc.sync.dma_start(out=w[:], in_=src[idx])
                    wts[key] = w

            def dma_w_half2(src, idx, key):
                w = wts[key]
                nc.sync.dma_start(out=w[:, C // 2:C], in_=src[idx][:, C // 2:C])

            def emit_tr_group(vts, kv, t4):
                """PE-transpose 4 [128,128] v tiles into one psum bank,
                then one DVE copy into vt."""
                trp = psP.tile([128, 512], bf16, name=f"tr_{kv}_{t4}", tag="p")
                for rr in range(4):
                    j = t4 * 4 + rr
                    nc.tensor.matmul(
                        trp[:, rr * 128:(rr + 1) * 128],
                        lhsT=vts[:, j * 128:(j + 1) * 128],
                        rhs=ident, is_transpose=True,
                        start=(rr == 0), stop=(rr == 3),
                        skip_group_check=True)
                nc.vector.tensor_copy(
                    out=vt[:, (kv * NT + t4 * 4) * 128:
                           (kv * NT + t4 * 4 + 4) * 128],
                    in_=trp[:])

            def startup():
                """q0+k0+v0 kc-major with 6 accumulators, per x half."""
                wq0, wk0, wv0 = wts["q0"], wts["k0"], wts["v0"]
                vts = vtsp.tile([128, T], bf16, name="vts_0", tag="vts")
                for half in (0, 1):
                    accq = [psA.tile([128, 512], f32, name=f"aq{half}{i}",
                                     tag="a") for i in (0, 1)]
                    acck = [psA.tile([128, 512], f32, name=f"ak{half}{i}",
                                     tag="a") for i in (0, 1)]
                    accv = [psS.tile([128, 512], f32, name=f"av{half}{i}",
                                     tag="s") for i in (0, 1)]
                    for kc in range(KC):
                        for acc, w in ((accq, wq0), (acck, wk0), (accv, wv0)):
                            for i in (0, 1):
                                nc.tensor.matmul(
                                    acc[i][:],
                                    lhsT=w[:, kc * 128:(kc + 1) * 128],
                                    rhs=xs(2 * half + i, kc),
                                    start=(kc == 0), stop=(kc == KC - 1))
                    for i in (0, 1):
                        t4 = 2 * half + i
                        nc.vector.tensor_copy(
                            out=qt[:, t4 * 512:(t4 + 1) * 512], in_=accq[i][:])
                        nc.vector.tensor_copy(
                            out=kt[:, t4 * 512:(t4 + 1) * 512], in_=acck[i][:])
                        nc.vector.tensor_copy(
                            out=vts[:, t4 * 512:(t4 + 1) * 512], in_=accv[i][:])
                    emit_tr_group(vts, 0, 2 * half)
                    emit_tr_group(vts, 0, 2 * half + 1)

            def proj_gen(wkey, dest, dbase, vts_kv=None, t4_order=(0, 1, 2, 3)):
                """t4-major projection of one [128, T] strip; yields at kc
                multiples of 8 for zipping."""
                w = wts[wkey]
                vts = None
                if vts_kv is not None:
                    vts = vtsp.tile([128, T], bf16, name=f"vts_{wkey}",
                                    tag="vts")
                for t4 in t4_order:
                    ps = psP.tile([128, 512], f32, name=f"ps_{wkey}_{t4}",
                                  tag="p")
                    for kc in range(KC):
                        nc.tensor.matmul(
                            ps[:], lhsT=w[:, kc * 128:(kc + 1) * 128],
                            rhs=xs(t4, kc),
                            start=(kc == 0), stop=(kc == KC - 1))
                        if kc % 8 == 7 and kc != KC - 1:
                            yield
                    if vts is None:
                        nc.vector.tensor_copy(
                            out=dest[:, dbase + t4 * 512:
                                     dbase + (t4 + 1) * 512],
                            in_=ps[:])
                    else:
                        nc.vector.tensor_copy(
                            out=vts[:, t4 * 512:(t4 + 1) * 512], in_=ps[:])
                        emit_tr_group(vts, vts_kv, t4)
                    yield

            def attn_gen(h, block_order=(0, 1, 2, 3)):
                kv = h % 2
                for b in block_order:
                    outT = psA.tile([128, 512], f32, name=f"oT_{h}_{b}",
                                    tag="a")
                    rs = psA.tile([128, 512], f32, name=f"rs_{h}_{b}", tag="a")
                    nj = 4 * b + 4
                    pts = [None] * 4
                    for j in range(nj):
                        r = j - 4 * b
                        roff = 128 * r if (TRIM and b > 0 and r > 0) else 0
                        sp = psS.tile([128, 512], f32, name=f"sp_{h}_{b}_{j}",
                                      tag="s")
                        nc.tensor.matmul(
                            sp[:, roff:512],
                            lhsT=kt[:, kv * T + j * 128:kv * T + (j + 1) * 128],
                            rhs=qt[:, h * T + b * 512 + roff:
                                   h * T + (b + 1) * 512],
                            start=True, stop=True)
                        pt = ptpool.tile([128, 512], bf16,
                                         name=f"pt_{h}_{b}_{j}", tag="pt")
                        nc.scalar.activation(pt[:, roff:512], sp[:, roff:512],
                                             EXP, scale=SCALE)
                        if r >= 0:
                            nc.vector.tensor_mul(
                                pt[:, roff:512], pt[:, roff:512],
                                mask_sb[:, r * 512 + roff:(r + 1) * 512])
                        nc.tensor.matmul(
                            outT[:, roff:512],
                            lhsT=vt[:, (kv * NT + j) * 128:
                                    (kv * NT + j + 1) * 128],
                            rhs=pt[:, roff:512],
                            start=(j == 0), stop=(j == nj - 1))
                        pts[j % 4] = (pt, roff)
                        if j == 0:
                            # full-array rowsum (0.25x) starts the rs bank
                            nc.tensor.matmul(
                                rs[:], lhsT=q128, rhs=pt[:, 0:512],
                                start=True, stop=False,
                                skip_group_check=True)
                        elif not PACK_RS:
                            nc.tensor.matmul(
                                rs[:, roff:512], lhsT=q128,
                                rhs=pt[:, roff:512],
                                start=False, stop=(j == nj - 1),
                                skip_group_check=True)
                        if j % 4 == 3:
                            if PACK_RS:
                                for rr in range(4):
                                    jj = j - 3 + rr
                                    if jj == 0:
                                        continue
                                    ptj, ro = pts[rr]
                                    nc.tensor.matmul(
                                        rs[32 * rr:32 * rr + 32, ro:512],
                                        lhsT=ones32, rhs=ptj[:, ro:512],
                                        tile_position=(0, 32 * rr),
                                        start=False, stop=(jj == nj - 1),
                                        skip_group_check=True)
                            yield
                    # drain: rowsum partials -> bcast-reduce -> recip -> mul
                    rsb = rsbp.tile([128, 512], bf16, name=f"rsb_{h}_{b}",
                                    tag="rsb")
                    nc.vector.tensor_copy(out=rsb[:], in_=rs[:])
                    bc = psS.tile([128, 512], f32, name=f"bc_{h}_{b}", tag="s")
                    nc.tensor.matmul(bc[:], lhsT=bcmask, rhs=rsb[:],
                                     start=True, stop=True)
                    rec = recp.tile([128, 512], f32, name=f"rec_{h}_{b}",
                                    tag="rec")
                    nc.vector.reciprocal(rec[:], bc[:])
                    off = ofp.tile([128, 512], f32, name=f"of_{h}_{b}",
                                   tag="of")
                    nc.vector.tensor_mul(off[:], outT[:], rec[:])
                    nc.sync.dma_start(
                        out=out_d[h * D:(h + 1) * D, b * 512:(b + 1) * 512],
                        in_=off[:])
                    yield

            def weave(*streams):
                """streams: (generator, weight) pairs; round-robin with
                weights until all exhausted."""
                live = [[g, wt] for g, wt in streams]
                while live:
                    done = []
                    for ent in live:
                        g, wt = ent
                        for _ in range(wt):
                            try:
                                next(g)
                            except StopIteration:
                                done.append(ent)
                                break
                    for ent in done:
                        live.remove(ent)

            def chain(*gens):
                for g in gens:
                    yield from g

            # ---- DMA schedule (ring is FIFO in issue order) ----
            dma_w(wq_d, 0, "q0", halves=True)
            dma_x(0)
            dma_w(wk_d, 0, "k0", halves=True)
            dma_x(1)
            dma_w(wv_d, 0, "v0", halves=True)
            dma_x(2)
            dma_w_half2(wq_d, 0, "q0")
            dma_w_half2(wk_d, 0, "k0")
            dma_w_half2(wv_d, 0, "v0")
            for c in range(3, 16):
                dma_x(c)

            # ---- compute schedule ----
            # An attn head must never weave with a unit it depends on (PE
            # stream is in-order; a dep on a later PE instruction deadlocks):
            #   attn0 needs q0,k0,v0 (startup); attn2 needs q2,k0,v0;
            #   attn1 needs q1,k1,v1; attn3 needs q3,k1,v1.
            startup()
            dma_w(wq_d, 2, "q2")
            dma_w(wk_d, 1, "k1")
            dma_w(wv_d, 1, "v1")
            weave((attn_gen(0), 1), (proj_gen("q2", qt, 2 * T), 1))
            dma_w(wq_d, 1, "q1")
            weave((attn_gen(2), 1),
                  (chain(proj_gen("k1", kt, T),
                         proj_gen("v1", None, 0, vts_kv=1),
                         proj_gen("q1", qt, T)), 3))
            dma_w(wq_d, 3, "q3")
            weave((attn_gen(1), 1),
                  (proj_gen("q3", qt, 3 * T, t4_order=(3, 2, 1, 0)), 1))
            weave((attn_gen(3, block_order=(3, 2, 1, 0)), 1))

    nc.compile()
    _prog_cache["nc"] = nc
    return nc


def _host_prep(x, Wq, bq, Wk, bk, Wv, bv):
    """Shard + repack inputs for the 8 cores. Returns in_maps list."""
    assert x.shape == (1, T, C)
    assert np.abs(bq).max() == 0 and np.abs(bk).max() == 0, \
        "nonzero q/k biases not supported"

    x0 = np.ascontiguousarray(x[0]).astype(BF16)
    # xh packed: [128, half*32K + kc*1024 + t'] = x[half*1024+t', kc*128+p]
    xh = np.ascontiguousarray(
        x0.reshape(2, 1024, KC, 128).transpose(3, 0, 2, 1).reshape(128, XCOLS))

    tq = np.arange(512)[None, :]
    tk = np.arange(128)[:, None]
    masks = np.concatenate(
        [(tq >= (tk + 128 * r)).astype(BF16) for r in range(4)]
        + [np.eye(128, dtype=BF16),
           np.ones((128, 32), dtype=BF16),
           np.full((128, 128), 0.25, dtype=BF16),
           np.full((128, 128), 1.0 / 32.0, dtype=BF16)], axis=1)
    masks = np.ascontiguousarray(masks)
    assert masks.shape == (128, MCOLS)

    def pack_w(Wrows):
        # Wrows: [128 (out c), C (in)] -> packed[p, 128*kc + c] =
        # Wrows[c, 128*kc + p]
        return np.ascontiguousarray(
            Wrows.astype(BF16).reshape(128, KC, 128).transpose(2, 1, 0)
            .reshape(128, C))

    in_maps = []
    for c in range(N_CORES):
        qheads = [2 * c, 2 * c + 1, 2 * c + 16, 2 * c + 17]
        kvheads = [2 * c, 2 * c + 1]
        wq = np.stack([pack_w(Wq[128 * H:128 * (H + 1)]) for H in qheads])
        wk = np.stack([pack_w(Wk[128 * K:128 * (K + 1)]) for K in kvheads])
        wv = np.stack([pack_w(Wv[128 * K:128 * (K + 1)]) for K in kvheads])
        in_maps.append({
            "xh": xh, "wq": wq, "wk": wk, "wv": wv, "masks": masks,
        })
    return in_maps


def _assemble(results, bv):
    out = np.empty((T, C), dtype=np.float32)
    for c in range(N_CORES):
        r = results[c]["out"]  # [NQH*D, T]
        qheads = [2 * c, 2 * c + 1, 2 * c + 16, 2 * c + 17]
        for i, H in enumerate(qheads):
            blk = r[128 * i:128 * (i + 1), :].T
            if bv is not None:
                blk = blk + bv[128 * (H % N_KV_HEADS):
                               128 * (H % N_KV_HEADS) + 128]
            out[:, 128 * H:128 * (H + 1)] = blk
    return out.reshape(1, T, C)


def _install_trace_hooks():
    """The agent image's antenv lacks axon_hooks; recreate it so
    run_bass_kernel_spmd's trace=True path can capture NTFF profiles."""
    import sys
    import types
    import antenv
    if "antenv.axon_hooks" not in sys.modules:
        mod = types.ModuleType("antenv.axon_hooks")
        mod._hook = None

        def set_axon_ntff_profile_hook(h):
            mod._hook = h

        def get_axon_ntff_profile_hook():
            return mod._hook

        mod.set_axon_ntff_profile_hook = set_axon_ntff_profile_hook
        mod.get_axon_ntff_profile_hook = get_axon_ntff_profile_hook
        sys.modules["antenv.axon_hooks"] = mod
        antenv.axon_hooks = mod
    from antenv.axon_hooks import (get_axon_ntff_profile_hook,
                                   set_axon_ntff_profile_hook)
    if get_axon_ntff_profile_hook() is None:
        if "/root/.axon_site" not in sys.path:
            sys.path.insert(0, "/root/.axon_site")
        from trn_agent_boot.trn_boot import _ntff_profile_via_ctypes
        set_axon_ntff_profile_hook(
            _ntff_profile_via_ctypes("/opt/axon/libaxon_pjrt.so"))
    import concourse.bass_utils as bu
    bu.upload_artifacts = lambda tmpdir: tmpdir


def _run(inputs, trace=False, trace_kwargs=None):
    if trace:
        _install_trace_hooks()
    from concourse.bass_utils import run_bass_kernel_spmd
    nc = _build_program()
    in_maps = _host_prep(**inputs)
    res = run_bass_kernel_spmd(
        nc, in_maps, list(range(N_CORES)), trace=trace,
        **(trace_kwargs or {}))
    bv = inputs["bv"].astype(np.float32)
    bv = bv if np.abs(bv).max() > 0 else None
    out = _assemble(res.results, bv)
    return out, res


def kernel(x, Wq, bq, Wk, bk, Wv, bv):
    out, _ = _run(dict(x=np.asarray(x), Wq=np.asarray(Wq), bq=np.asarray(bq),
                       Wk=np.asarray(Wk), bk=np.asarray(bk),
                       Wv=np.asarray(Wv), bv=np.asarray(bv)))
    return out


# revision 6
# speedup vs baseline: 1.1846x; 1.1846x over previous
"""GQA kernel for Trainium2, 8 NeuronCores, tensor-parallel over heads.

Problem: B=1, T=2048, C=4096, 32 q-heads, 16 kv-heads, head_dim=128,
scale = 1/sqrt(32), causal. q head H uses kv head H%16.

Sharding (no collectives needed): core c owns q-heads
{2c, 2c+1, 2c+16, 2c+17} and kv-heads {2c, 2c+1}. Each output column
block depends only on its own head, so the full output is a host-side
concat of per-core column slices.

Per-core kernel (all matmuls bf16, fp32 PSUM accumulation):
  x resident in SBUF as [128, half*32K + kc*1024 + t'] (16 1MB chunk
  tiles; DMA order interleaves the first weight halves with the first
  x chunks so projection matmuls start as soon as chunk 0 lands).
  Startup: q0+k0+v0 projected kc-major with 6 psum accumulators
  (t4 0,1 then 2,3), tracking x-chunk arrival; later units t4-major.
  v strips are PE-transposed into vt ([tk,129] tiles with a ones
  column for the row-sum trick); 4 transposes share one psum bank
  (single-start trick) and drain with one strided DVE copy.
  Attention per head (4 Tq blocks of 512, Tk pairs of 2x128):
    S^T pair = kt^T @ qt -> [128,1024] PSUM, exp (ACT, scale folded),
    causal mask via {0,1} multiply on diagonal tiles (trimmed free
    dims on diagonal pairs), PV: pt slices as stationary, rhs v
    [tk,129]; out normalized by reciprocal(row-sum col).
  attn3 runs blocks 3..0 after q3's t4 3..0 so the kernel tail is the
  smallest block.
"""

import numpy as np
import ml_dtypes

BF16 = ml_dtypes.bfloat16
T = 2048
C = 4096
D = 128
N_HEADS = 32
N_KV_HEADS = 16
SCALE = float(1.0 / np.sqrt(np.float32(N_HEADS)))
KC = C // 128          # 32 contraction chunks
NQH = 4                # local q heads per core
NKV = 2                # local kv heads per core
NT = T // 128          # 16 token tiles
VROW = D + 1           # 129: v with ones column
N_CORES = 8
XCOLS = 2 * KC * 1024

_prog_cache = {}


def _build_program():
    if "nc" in _prog_cache:
        return _prog_cache["nc"]
    import concourse.bass as bass
    import concourse.tile as tile
    from concourse import bacc, mybir

    dt = mybir.dt
    f32 = dt.float32
    bf16 = dt.bfloat16
    EXP = mybir.ActivationFunctionType.Exp

    nc = bacc.Bacc("TRN2", target_bir_lowering=False, debug=False,
                   num_devices=N_CORES)

    xh_d = nc.dram_tensor("xh", [128, XCOLS], bf16, kind="ExternalInput").ap()
    wq_d = nc.dram_tensor("wq", [NQH, 128, C], bf16, kind="ExternalInput").ap()
    wk_d = nc.dram_tensor("wk", [NKV, 128, C], bf16, kind="ExternalInput").ap()
    wv_d = nc.dram_tensor("wv", [NKV, 128, C], bf16, kind="ExternalInput").ap()
    # masks: 4x [128,512] causal tiles + [128,128] identity for PE transpose
    mask_d = nc.dram_tensor("masks", [128, 4 * 512 + 128], bf16,
                            kind="ExternalInput").ap()
    out_d = nc.dram_tensor("out", [T, NQH * D], f32, kind="ExternalOutput").ap()

    with tile.TileContext(nc) as tc:
        with (
            tc.tile_pool(name="persist", bufs=1) as persist,
            tc.tile_pool(name="wpool", bufs=3) as wpool,
            tc.tile_pool(name="vtsp", bufs=1) as vtsp,
            tc.tile_pool(name="ptpool", bufs=3) as ptpool,
            tc.tile_pool(name="opool", bufs=4) as opool,
            tc.tile_pool(name="recpool", bufs=4) as recpool,
            tc.tile_pool(name="psum", bufs=4, space=bass.MemorySpace.PSUM) as psum,
            tc.tile_pool(name="psum2", bufs=2, space=bass.MemorySpace.PSUM) as psum2,
        ):
            mask_sb = persist.tile([128, 4 * 512 + 128], bf16, name="mask_sb",
                                   tag="mask_sb")
            nc.sync.dma_start(out=mask_sb[:], in_=mask_d[:])
            ident = mask_sb[:, 4 * 512: 4 * 512 + 128]

            qt = persist.tile([128, NQH * T], bf16, name="qt", tag="qt")
            kt = persist.tile([128, NKV * T], bf16, name="kt", tag="kt")
            vt = persist.tile([128, NKV * NT * VROW], bf16, name="vt", tag="vt")

            # ones columns of v (row-sum trick)
            for i in range(NKV * NT):
                nc.vector.memset(vt[:, i * VROW + D: (i + 1) * VROW], 1.0)

            xcs = [None] * 16
            wts = {}

            def dma_x(c):
                xc = persist.tile([128, 4096], bf16, name=f"xc{c}",
                                  tag=f"xc{c}")
                nc.sync.dma_start(out=xc[:], in_=xh_d[:, c * 4096:(c + 1) * 4096])
                xcs[c] = xc

            def xs(t4, kc):
                c = (t4 // 2) * 8 + kc // 4
                off = (kc % 4) * 1024 + (t4 % 2) * 512
                return xcs[c][:, off:off + 512]

            def dma_w(src, idx, key, half=None):
                if half is None or half == 0:
                    w = wpool.tile([128, C], bf16, name=f"w_{key}", tag="w")
                    wts[key] = w
                w = wts[key]
                if half is None:
                    nc.sync.dma_start(out=w[:], in_=src[idx])
                elif half == 0:
                    nc.sync.dma_start(out=w[:, 0:C // 2],
                                      in_=src[idx][:, 0:C // 2])
                else:
                    nc.sync.dma_start(out=w[:, C // 2:C],
                                      in_=src[idx][:, C // 2:C])

            def emit_tr_group(vts, kv, t4):
                """PE-transpose 4 [128,128] v tiles into one psum bank
                (single-start trick), one strided DVE copy into vt."""
                trp = psum.tile([128, 512], bf16, name=f"tr_{kv}_{t4}",
                                tag="ps")
                for rr in range(4):
                    j = t4 * 4 + rr
                    nc.tensor.matmul(
                        trp[:, rr * 128:(rr + 1) * 128],
                        lhsT=vts[:, j * 128:(j + 1) * 128],
                        rhs=ident, is_transpose=True,
                        start=(rr == 0), stop=(rr == 3),
                        skip_group_check=True)
                for rr in range(4):
                    j = t4 * 4 + rr
                    nc.vector.tensor_copy(
                        out=vt[:, (kv * NT + j) * VROW:
                               (kv * NT + j) * VROW + D],
                        in_=trp[:, rr * 128:(rr + 1) * 128])

            def startup():
                """q0+k0+v0 kc-major with 6 accumulators, per x half."""
                wq0, wk0, wv0 = wts["q0"], wts["k0"], wts["v0"]
                vts = vtsp.tile([128, T], bf16, name="vts_0", tag="vts")
                for half in (0, 1):
                    aq = psum2.tile([128, 1024], f32, name=f"aq{half}",
                                    tag="sp2")
                    accq = [aq[:, 0:512], aq[:, 512:1024]]
                    acck = [psum.tile([128, 512], f32, name=f"ak{half}{i}",
                                      tag="ps")[:] for i in (0, 1)]
                    accv = [psum.tile([128, 512], f32, name=f"av{half}{i}",
                                      tag="ps")[:] for i in (0, 1)]
                    for kc in range(KC):
                        for acc, w in ((accq, wq0), (acck, wk0), (accv, wv0)):
                            for i in (0, 1):
                                nc.tensor.matmul(
                                    acc[i],
                                    lhsT=w[:, kc * 128:(kc + 1) * 128],
                                    rhs=xs(2 * half + i, kc),
                                    start=(kc == 0), stop=(kc == KC - 1))
                    for i in (0, 1):
                        t4 = 2 * half + i
                        nc.vector.tensor_copy(
                            out=qt[:, t4 * 512:(t4 + 1) * 512], in_=accq[i])
                        nc.vector.tensor_copy(
                            out=kt[:, t4 * 512:(t4 + 1) * 512], in_=acck[i])
                        nc.vector.tensor_copy(
                            out=vts[:, t4 * 512:(t4 + 1) * 512], in_=accv[i])
                    emit_tr_group(vts, 0, 2 * half)
                    emit_tr_group(vts, 0, 2 * half + 1)

            def proj(wkey, dest, dbase, vts_kv=None, t4_order=(0, 1, 2, 3)):
                """t4-major projection of one [128, T] strip."""
                w = wts[wkey]
                vts = None
                if vts_kv is not None:
                    vts = vtsp.tile([128, T], bf16, name=f"vts_{wkey}",
                                    tag="vts")
                with nc.named_scope(f"proj_{wkey}"):
                    for t4 in t4_order:
                        ps = psum.tile([128, 512], f32,
                                       name=f"ps_{wkey}_{t4}", tag="ps")
                        for kc in range(KC):
                            nc.tensor.matmul(
                                ps[:], lhsT=w[:, kc * 128:(kc + 1) * 128],
                                rhs=xs(t4, kc),
                                start=(kc == 0), stop=(kc == KC - 1))
                        if vts is None:
                            nc.vector.tensor_copy(
                                out=dest[:, dbase + t4 * 512:
                                         dbase + (t4 + 1) * 512],
                                in_=ps[:])
                        else:
                            nc.vector.tensor_copy(
                                out=vts[:, t4 * 512:(t4 + 1) * 512], in_=ps[:])
                            emit_tr_group(vts, vts_kv, t4)

            def attn(h, block_order=(0, 1, 2, 3)):
                kv = h % 2
                with nc.named_scope(f"attn_{h}"):
                    for b in block_order:
                        pvs = []
                        for s in range(4):
                            pv = psum.tile([128, 512], f32,
                                           name=f"pv_{h}_{b}_{s}", tag="ps")
                            pvs.append(pv)
                        for p in range(2 * b + 2):  # pairs of Tk tiles
                            diag = p >= 2 * b
                            spp = psum2.tile([128, 1024], f32,
                                             name=f"sp_{h}_{b}_{p}", tag="sp2")
                            pt = ptpool.tile([128, 1024], bf16,
                                             name=f"pt_{h}_{b}_{p}", tag="pt")
                            qsl = h * T + b * 512
                            for half in range(2):
                                j = 2 * p + half
                                r = j - 4 * b
                                roff = 128 * r if (diag and r > 0) else 0
                                nc.tensor.matmul(
                                    spp[:, half * 512 + roff:
                                        (half + 1) * 512],
                                    lhsT=kt[:, kv * T + j * 128:
                                            kv * T + (j + 1) * 128],
                                    rhs=qt[:, qsl + roff:qsl + 512],
                                    start=True, stop=True,
                                )
                                if diag:
                                    nc.scalar.activation(
                                        pt[:, half * 512 + roff:
                                           (half + 1) * 512],
                                        spp[:, half * 512 + roff:
                                            (half + 1) * 512],
                                        EXP, scale=SCALE)
                                    if r >= 0:
                                        nc.vector.tensor_mul(
                                            pt[:, half * 512 + roff:
                                               (half + 1) * 512],
                                            pt[:, half * 512 + roff:
                                               (half + 1) * 512],
                                            mask_sb[:, r * 512 + roff:
                                                    (r + 1) * 512])
                            if not diag:
                                nc.scalar.activation(pt[:], spp[:], EXP,
                                                     scale=SCALE)
                            for half in range(2):
                                j = 2 * p + half
                                r = j - 4 * b
                                vsl = vt[:, (kv * NT + j) * VROW:
                                         (kv * NT + j + 1) * VROW]
                                for s in range(max(0, r), 4):
                                    nc.tensor.matmul(
                                        pvs[s][:, 0:VROW],
                                        lhsT=pt[:, half * 512 + s * 128:
                                                half * 512 + (s + 1) * 128],
                                        rhs=vsl,
                                        start=(j == 0), stop=(j == 4 * b + s),
                                    )
                        for s in range(4):
                            rec = recpool.tile([128, 1], f32,
                                               name=f"rec_{h}_{b}_{s}",
                                               tag="rec")
                            nc.vector.reciprocal(rec[:], pvs[s][:, D:D + 1])
                            ot = opool.tile([128, 128], f32,
                                            name=f"ot_{h}_{b}_{s}", tag="ot")
                            nc.vector.tensor_scalar_mul(ot[:], pvs[s][:, 0:D],
                                                        rec[:])
                            nc.sync.dma_start(
                                out=out_d[b * 512 + s * 128:
                                          b * 512 + (s + 1) * 128,
                                          h * D:(h + 1) * D],
                                in_=ot[:])

            # ---- DMA schedule (ring is FIFO in issue order) ----
            dma_w(wq_d, 0, "q0", half=0)
            dma_x(0)
            dma_w(wk_d, 0, "k0", half=0)
            dma_x(1)
            dma_w(wv_d, 0, "v0", half=0)
            dma_x(2)
            dma_w(wq_d, 0, "q0", half=1)
            dma_w(wk_d, 0, "k0", half=1)
            dma_w(wv_d, 0, "v0", half=1)
            for c in range(3, 16):
                dma_x(c)

            # ---- compute schedule ----
            startup()
            dma_w(wq_d, 2, "q2")
            dma_w(wk_d, 1, "k1")
            dma_w(wv_d, 1, "v1")
            attn(0)
            proj("q2", qt, 2 * T)
            dma_w(wq_d, 1, "q1")
            attn(2)
            proj("k1", kt, T)
            dma_w(wq_d, 3, "q3")
            proj("v1", None, 0, vts_kv=1)
            proj("q1", qt, T)
            attn(1)
            proj("q3", qt, 3 * T, t4_order=(3, 2, 1, 0))
            attn(3, block_order=(3, 2, 1, 0))

    nc.compile()
    _prog_cache["nc"] = nc
    return nc


def _host_prep(x, Wq, bq, Wk, bk, Wv, bv):
    """Shard + repack inputs for the 8 cores. Returns in_maps list."""
    assert x.shape == (1, T, C)
    assert np.abs(bq).max() == 0 and np.abs(bk).max() == 0, \
        "nonzero q/k biases not supported"

    x0 = np.ascontiguousarray(x[0]).astype(BF16)
    # xh packed: [128, half*32K + kc*1024 + t'] = x[half*1024+t', kc*128+p]
    xh = np.ascontiguousarray(
        x0.reshape(2, 1024, KC, 128).transpose(3, 0, 2, 1).reshape(128, XCOLS))

    # causal masks for the 4 diagonal-tile offsets: mask_r[tk,tq] = tq >= tk+128r
    tq = np.arange(512)[None, :]
    tk = np.arange(128)[:, None]
    masks = np.concatenate(
        [(tq >= (tk + 128 * r)).astype(BF16) for r in range(4)]
        + [np.eye(128, dtype=BF16)], axis=1)
    masks = np.ascontiguousarray(masks)

    def pack_w(Wrows):
        # Wrows: [128 (out c), C (in)] for one head ->
        # packed[p, 128*kc + c] = Wrows[c, 128*kc + p]
        return np.ascontiguousarray(
            Wrows.astype(BF16).reshape(128, KC, 128).transpose(2, 1, 0)
            .reshape(128, C))

    in_maps = []
    for c in range(N_CORES):
        qheads = [2 * c, 2 * c + 1, 2 * c + 16, 2 * c + 17]
        kvheads = [2 * c, 2 * c + 1]
        wq = np.stack([pack_w(Wq[128 * H:128 * (H + 1)]) for H in qheads])
        wk = np.stack([pack_w(Wk[128 * K:128 * (K + 1)]) for K in kvheads])
        wv = np.stack([pack_w(Wv[128 * K:128 * (K + 1)]) for K in kvheads])
        in_maps.append({
            "xh": xh, "wq": wq, "wk": wk, "wv": wv, "masks": masks,
        })
    return in_maps


def _assemble(results, bv):
    out = np.empty((T, C), dtype=np.float32)
    for c in range(N_CORES):
        r = results[c]["out"]
        qheads = [2 * c, 2 * c + 1, 2 * c + 16, 2 * c + 17]
        for i, H in enumerate(qheads):
            blk = r[:, 128 * i:128 * (i + 1)]
            if bv is not None:
                blk = blk + bv[128 * (H % N_KV_HEADS):
                               128 * (H % N_KV_HEADS) + 128]
            out[:, 128 * H:128 * (H + 1)] = blk
    return out.reshape(1, T, C)


def _install_trace_hooks():
    """The agent image's antenv lacks axon_hooks; recreate it so
    run_bass_kernel_spmd's trace=True path can capture NTFF profiles."""
    import sys
    import types
    import antenv
    if "antenv.axon_hooks" not in sys.modules:
        mod = types.ModuleType("antenv.axon_hooks")
        mod._hook = None

        def set_axon_ntff_profile_hook(h):
            mod._hook = h

        def get_axon_ntff_profile_hook():
            return mod._hook

        mod.set_axon_ntff_profile_hook = set_axon_ntff_profile_hook
        mod.get_axon_ntff_profile_hook = get_axon_ntff_profile_hook
        sys.modules["antenv.axon_hooks"] = mod
        antenv.axon_hooks = mod
    from antenv.axon_hooks import (get_axon_ntff_profile_hook,
                                   set_axon_ntff_profile_hook)
    if get_axon_ntff_profile_hook() is None:
        if "/root/.axon_site" not in sys.path:
            sys.path.insert(0, "/root/.axon_site")
        from trn_agent_boot.trn_boot import _ntff_profile_via_ctypes
        set_axon_ntff_profile_hook(
            _ntff_profile_via_ctypes("/opt/axon/libaxon_pjrt.so"))
    import concourse.bass_utils as bu
    bu.upload_artifacts = lambda tmpdir: tmpdir


def _run(inputs, trace=False, trace_kwargs=None):
    if trace:
        _install_trace_hooks()
    from concourse.bass_utils import run_bass_kernel_spmd
    nc = _build_program()
    in_maps = _host_prep(**inputs)
    res = run_bass_kernel_spmd(
        nc, in_maps, list(range(N_CORES)), trace=trace,
        **(trace_kwargs or {}))
    bv = inputs["bv"].astype(np.float32)
    bv = bv if np.abs(bv).max() > 0 else None
    out = _assemble(res.results, bv)
    return out, res


def kernel(x, Wq, bq, Wk, bk, Wv, bv):
    out, _ = _run(dict(x=np.asarray(x), Wq=np.asarray(Wq), bq=np.asarray(bq),
                       Wk=np.asarray(Wk), bk=np.asarray(bk),
                       Wv=np.asarray(Wv), bv=np.asarray(bv)))
    return out


# revision 12
# speedup vs baseline: 1.1984x; 1.0116x over previous
"""GQA kernel for Trainium2, 8 NeuronCores, tensor-parallel over heads.

Problem: B=1, T=2048, C=4096, 32 q-heads, 16 kv-heads, head_dim=128,
scale = 1/sqrt(32), causal. q head H uses kv head H%16.

Sharding (no collectives needed): core c owns q-heads
{2c, 2c+1, 2c+16, 2c+17} and kv-heads {2c, 2c+1}. Each output column
block depends only on its own head, so the full output is a host-side
concat of per-core column slices.

Per-core kernel (all matmuls bf16, fp32 PSUM accumulation):
  x resident in SBUF as [128, half*32K + kc*1024 + t'] (16 1MB chunk
  tiles; DMA order interleaves the first weight halves with the first
  x chunks so projection matmuls start as soon as chunk 0 lands).
  Startup: q0+k0+v0 projected kc-major with 6 psum accumulators
  (t4 0,1 then 2,3), tracking x-chunk arrival; later units t4-major.
  v strips are PE-transposed into vt ([tk,129] tiles with a ones
  column for the row-sum trick); 4 transposes share one psum bank
  (single-start trick) and drain with one strided DVE copy.
  Attention per head (4 Tq blocks of 512, Tk pairs of 2x128):
    S^T pair = kt^T @ qt -> [128,1024] PSUM, exp (ACT, scale folded),
    causal mask via {0,1} multiply on diagonal tiles (trimmed free
    dims on diagonal pairs), PV: pt slices as stationary, rhs v
    [tk,129]; out normalized by reciprocal(row-sum col).
  attn3 runs blocks 3..0 after q3's t4 3..0 so the kernel tail is the
  smallest block.
"""

import numpy as np
import ml_dtypes

BF16 = ml_dtypes.bfloat16
T = 2048
C = 4096
D = 128
N_HEADS = 32
N_KV_HEADS = 16
SCALE = float(1.0 / np.sqrt(np.float32(N_HEADS)))
KC = C // 128          # 32 contraction chunks
NQH = 4                # local q heads per core
NKV = 2                # local kv heads per core
NT = T // 128          # 16 token tiles
VROW = D + 1           # 129: v with ones column
N_CORES = 8
XCOLS = 2 * KC * 1024

_prog_cache = {}


def _build_program():
    if "nc" in _prog_cache:
        return _prog_cache["nc"]
    import concourse.bass as bass
    import concourse.tile as tile
    from concourse import bacc, mybir

    dt = mybir.dt
    f32 = dt.float32
    bf16 = dt.bfloat16
    EXP = mybir.ActivationFunctionType.Exp

    nc = bacc.Bacc("TRN2", target_bir_lowering=False, debug=False,
                   num_devices=N_CORES)

    xh_d = nc.dram_tensor("xh", [128, XCOLS], bf16, kind="ExternalInput").ap()
    wq_d = nc.dram_tensor("wq", [NQH, 128, C], bf16, kind="ExternalInput").ap()
    wk_d = nc.dram_tensor("wk", [NKV, 128, C], bf16, kind="ExternalInput").ap()
    wv_d = nc.dram_tensor("wv", [NKV, 128, C], bf16, kind="ExternalInput").ap()
    # masks: 4x [128,512] causal tiles + [128,128] identity for PE transpose
    mask_d = nc.dram_tensor("masks", [128, 4 * 512 + 128], bf16,
                            kind="ExternalInput").ap()
    out_d = nc.dram_tensor("out", [T, NQH * D], f32, kind="ExternalOutput").ap()

    with tile.TileContext(nc) as tc:
        with (
            tc.tile_pool(name="persist", bufs=1) as persist,
            tc.tile_pool(name="wpool", bufs=3) as wpool,
            tc.tile_pool(name="vtsp", bufs=1) as vtsp,
            tc.tile_pool(name="ptpool", bufs=3) as ptpool,
            tc.tile_pool(name="opool", bufs=4) as opool,
            tc.tile_pool(name="recpool", bufs=4) as recpool,
            tc.tile_pool(name="psum", bufs=4, space=bass.MemorySpace.PSUM) as psum,
            tc.tile_pool(name="psum2", bufs=2, space=bass.MemorySpace.PSUM) as psum2,
        ):
            mask_sb = persist.tile([128, 4 * 512 + 128], bf16, name="mask_sb",
                                   tag="mask_sb")
            ident = mask_sb[:, 4 * 512: 4 * 512 + 128]

            qt = persist.tile([128, NQH * T], bf16, name="qt", tag="qt")
            kt = persist.tile([128, NKV * T], bf16, name="kt", tag="kt")
            vt = persist.tile([128, NKV * NT * VROW], bf16, name="vt", tag="vt")

            # ones columns of v (row-sum trick)
            for i in range(NKV * NT):
                nc.vector.memset(vt[:, i * VROW + D: (i + 1) * VROW], 1.0)

            xcs = [None] * 16
            wts = {}

            def dma_x(c):
                xc = persist.tile([128, 4096], bf16, name=f"xc{c}",
                                  tag=f"xc{c}")
                nc.sync.dma_start(out=xc[:], in_=xh_d[:, c * 4096:(c + 1) * 4096])
                xcs[c] = xc

            def xs(t4, kc):
                c = (t4 // 2) * 8 + kc // 4
                off = (kc % 4) * 1024 + (t4 % 2) * 512
                return xcs[c][:, off:off + 512]

            def dma_w(src, idx, key, cols=None):
                if key not in wts:
                    w = wpool.tile([128, C], bf16, name=f"w_{key}", tag="w")
                    wts[key] = w
                w = wts[key]
                if cols is None:
                    nc.sync.dma_start(out=w[:], in_=src[idx])
                else:
                    lo, hi = cols
                    nc.sync.dma_start(out=w[:, lo:hi], in_=src[idx][:, lo:hi])

            def emit_tr_group(vts, kv, t4):
                """PE-transpose 4 [128,128] v tiles into one psum bank
                (single-start trick), one strided DVE copy into vt."""
                trp = psum.tile([128, 512], bf16, name=f"tr_{kv}_{t4}",
                                tag="ps")
                for rr in range(4):
                    j = t4 * 4 + rr
                    nc.tensor.matmul(
                        trp[:, rr * 128:(rr + 1) * 128],
                        lhsT=vts[:, j * 128:(j + 1) * 128],
                        rhs=ident, is_transpose=True,
                        start=(rr == 0), stop=(rr == 3),
                        skip_group_check=True)
                for rr in range(4):
                    j = t4 * 4 + rr
                    nc.vector.tensor_copy(
                        out=vt[:, (kv * NT + j) * VROW:
                               (kv * NT + j) * VROW + D],
                        in_=trp[:, rr * 128:(rr + 1) * 128])

            def startup():
                """q0+k0+v0 kc-major with 6 accumulators, per x half."""
                wq0, wk0, wv0 = wts["q0"], wts["k0"], wts["v0"]
                vts = vtsp.tile([128, T], bf16, name="vts_0", tag="vts")
                for half in (0, 1):
                    aq = psum2.tile([128, 1024], f32, name=f"aq{half}",
                                    tag="sp2")
                    accq = [aq[:, 0:512], aq[:, 512:1024]]
                    acck = [psum.tile([128, 512], f32, name=f"ak{half}{i}",
                                      tag="ps")[:] for i in (0, 1)]
                    accv = [psum.tile([128, 512], f32, name=f"av{half}{i}",
                                      tag="ps")[:] for i in (0, 1)]
                    for kc in range(KC):
                        for acc, w in ((accq, wq0), (acck, wk0), (accv, wv0)):
                            for i in (0, 1):
                                nc.tensor.matmul(
                                    acc[i],
                                    lhsT=w[:, kc * 128:(kc + 1) * 128],
                                    rhs=xs(2 * half + i, kc),
                                    start=(kc == 0), stop=(kc == KC - 1))
                    for i in (0, 1):
                        t4 = 2 * half + i
                        nc.vector.tensor_copy(
                            out=qt[:, t4 * 512:(t4 + 1) * 512], in_=accq[i])
                        nc.vector.tensor_copy(
                            out=kt[:, t4 * 512:(t4 + 1) * 512], in_=acck[i])
                        nc.vector.tensor_copy(
                            out=vts[:, t4 * 512:(t4 + 1) * 512], in_=accv[i])
                    emit_tr_group(vts, 0, 2 * half)
                    emit_tr_group(vts, 0, 2 * half + 1)

            def proj(wkey, dest, dbase, vts_kv=None, t4_order=(0, 1, 2, 3)):
                """t4-major projection of one [128, T] strip."""
                w = wts[wkey]
                vts = None
                if vts_kv is not None:
                    vts = vtsp.tile([128, T], bf16, name=f"vts_{wkey}",
                                    tag="vts")
                with nc.named_scope(f"proj_{wkey}"):
                    for t4 in t4_order:
                        ps = psum.tile([128, 512], f32,
                                       name=f"ps_{wkey}_{t4}", tag="ps")
                        for kc in range(KC):
                            nc.tensor.matmul(
                                ps[:], lhsT=w[:, kc * 128:(kc + 1) * 128],
                                rhs=xs(t4, kc),
                                start=(kc == 0), stop=(kc == KC - 1))
                        if vts is None:
                            nc.vector.tensor_copy(
                                out=dest[:, dbase + t4 * 512:
                                         dbase + (t4 + 1) * 512],
                                in_=ps[:])
                        else:
                            nc.vector.tensor_copy(
                                out=vts[:, t4 * 512:(t4 + 1) * 512], in_=ps[:])
                    if vts is not None:
                        for t4 in range(4):
                            emit_tr_group(vts, vts_kv, t4)

            def attn(h, block_order=(0, 1, 2, 3)):
                kv = h % 2
                with nc.named_scope(f"attn_{h}"):
                    for b in block_order:
                        pvs = []
                        for s in range(4):
                            pv = psum.tile([128, 512], f32,
                                           name=f"pv_{h}_{b}_{s}", tag="ps")
                            pvs.append(pv)
                        for p in range(2 * b + 2):  # pairs of Tk tiles
                            spp = psum2.tile([128, 1024], f32,
                                             name=f"sp_{h}_{b}_{p}", tag="sp2")
                            pt = ptpool.tile([128, 1024], bf16,
                                             name=f"pt_{h}_{b}_{p}", tag="pt")
                            qsl = qt[:, h * T + b * 512:h * T + (b + 1) * 512]
                            for half in range(2):
                                j = 2 * p + half
                                nc.tensor.matmul(
                                    spp[:, half * 512:(half + 1) * 512],
                                    lhsT=kt[:, kv * T + j * 128:
                                            kv * T + (j + 1) * 128],
                                    rhs=qsl,
                                    start=True, stop=True,
                                )
                            nc.scalar.activation(pt[:], spp[:], EXP,
                                                 scale=SCALE)
                            if p >= 2 * b:  # diagonal pairs, mask-strip aligned
                                roff = (p - 2 * b) * 1024
                                nc.vector.tensor_mul(
                                    pt[:], pt[:],
                                    mask_sb[:, roff:roff + 1024])
                            for half in range(2):
                                j = 2 * p + half
                                r = j - 4 * b
                                vsl = vt[:, (kv * NT + j) * VROW:
                                         (kv * NT + j + 1) * VROW]
                                for s in range(max(0, r), 4):
                                    nc.tensor.matmul(
                                        pvs[s][:, 0:VROW],
                                        lhsT=pt[:, half * 512 + s * 128:
                                                half * 512 + (s + 1) * 128],
                                        rhs=vsl,
                                        start=(j == 0), stop=(j == 4 * b + s),
                                    )
                        for s in range(4):
                            rec = recpool.tile([128, 1], f32,
                                               name=f"rec_{h}_{b}_{s}",
                                               tag="rec")
                            nc.vector.reciprocal(rec[:], pvs[s][:, D:D + 1])
                            ot = opool.tile([128, 128], f32,
                                            name=f"ot_{h}_{b}_{s}", tag="ot")
                            nc.vector.tensor_scalar_mul(ot[:], pvs[s][:, 0:D],
                                                        rec[:])
                            nc.sync.dma_start(
                                out=out_d[b * 512 + s * 128:
                                          b * 512 + (s + 1) * 128,
                                          h * D:(h + 1) * D],
                                in_=ot[:])

            # ---- DMA schedule (ring is FIFO in issue order) ----
            # First weight quarters (kc 0-7) for all three startup units,
            # then x chunks stream; startup compute is PE-bound from the
            # first chunk.
            Q = C // 4
            for key, src in (("q0", wq_d), ("k0", wk_d), ("v0", wv_d)):
                dma_w(src, 0 if key == "q0" else 0, key, cols=(0, Q))
            dma_x(0)
            dma_x(1)
            for key, src in (("q0", wq_d), ("k0", wk_d), ("v0", wv_d)):
                dma_w(src, 0, key, cols=(Q, 2 * Q))
            dma_x(2)
            dma_x(3)
            for key, src in (("q0", wq_d), ("k0", wk_d), ("v0", wv_d)):
                dma_w(src, 0, key, cols=(2 * Q, C))
            nc.sync.dma_start(out=mask_sb[:], in_=mask_d[:])
            for c in range(4, 16):
                dma_x(c)

            # ---- compute schedule ----
            startup()
            dma_w(wq_d, 2, "q2")
            dma_w(wk_d, 1, "k1")
            dma_w(wv_d, 1, "v1")
            attn(0)
            proj("q2", qt, 2 * T)
            dma_w(wq_d, 1, "q1")
            attn(2)
            proj("k1", kt, T)
            dma_w(wq_d, 3, "q3")
            proj("v1", None, 0, vts_kv=1)
            proj("q1", qt, T)
            attn(1)
            proj("q3", qt, 3 * T)
            attn(3)

    nc.compile()
    _prog_cache["nc"] = nc
    return nc


def _host_prep(x, Wq, bq, Wk, bk, Wv, bv):
    """Shard + repack inputs for the 8 cores. Returns in_maps list."""
    assert x.shape == (1, T, C)
    assert np.abs(bq).max() == 0 and np.abs(bk).max() == 0, \
        "nonzero q/k biases not supported"

    x0 = np.ascontiguousarray(x[0]).astype(BF16)
    # xh packed: [128, half*32K + kc*1024 + t'] = x[half*1024+t', kc*128+p]
    xh = np.ascontiguousarray(
        x0.reshape(2, 1024, KC, 128).transpose(3, 0, 2, 1).reshape(128, XCOLS))

    # causal masks for the 4 diagonal-tile offsets: mask_r[tk,tq] = tq >= tk+128r
    tq = np.arange(512)[None, :]
    tk = np.arange(128)[:, None]
    masks = np.concatenate(
        [(tq >= (tk + 128 * r)).astype(BF16) for r in range(4)]
        + [np.eye(128, dtype=BF16)], axis=1)
    masks = np.ascontiguousarray(masks)

    def pack_w(Wrows):
        # Wrows: [128 (out c), C (in)] for one head ->
        # packed[p, 128*kc + c] = Wrows[c, 128*kc + p]
        return np.ascontiguousarray(
            Wrows.astype(BF16).reshape(128, KC, 128).transpose(2, 1, 0)
            .reshape(128, C))

    in_maps = []
    for c in range(N_CORES):
        qheads = [2 * c, 2 * c + 1, 2 * c + 16, 2 * c + 17]
        kvheads = [2 * c, 2 * c + 1]
        wq = np.stack([pack_w(Wq[128 * H:128 * (H + 1)]) for H in qheads])
        wk = np.stack([pack_w(Wk[128 * K:128 * (K + 1)]) for K in kvheads])
        wv = np.stack([pack_w(Wv[128 * K:128 * (K + 1)]) for K in kvheads])
        in_maps.append({
            "xh": xh, "wq": wq, "wk": wk, "wv": wv, "masks": masks,
        })
    return in_maps


def _assemble(results, bv):
    out = np.empty((T, C), dtype=np.float32)
    for c in range(N_CORES):
        r = results[c]["out"]
        qheads = [2 * c, 2 * c + 1, 2 * c + 16, 2 * c + 17]
        for i, H in enumerate(qheads):
            blk = r[:, 128 * i:128 * (i + 1)]
            if bv is not None:
                blk = blk + bv[128 * (H % N_KV_HEADS):
                               128 * (H % N_KV_HEADS) + 128]
            out[:, 128 * H:128 * (H + 1)] = blk
    return out.reshape(1, T, C)


def _install_trace_hooks():
    """The agent image's antenv lacks axon_hooks; recreate it so
    run_bass_kernel_spmd's trace=True path can capture NTFF profiles."""
    import sys
    import types
    import antenv
    if "antenv.axon_hooks" not in sys.modules:
        mod = types.ModuleType("antenv.axon_hooks")
        mod._hook = None

        def set_axon_ntff_profile_hook(h):
            mod._hook = h

        def get_axon_ntff_profile_hook():
            return mod._hook

        mod.set_axon_ntff_profile_hook = set_axon_ntff_profile_hook
        mod.get_axon_ntff_profile_hook = get_axon_ntff_profile_hook
        sys.modules["antenv.axon_hooks"] = mod
        antenv.axon_hooks = mod
    from antenv.axon_hooks import (get_axon_ntff_profile_hook,
                                   set_axon_ntff_profile_hook)
    if get_axon_ntff_profile_hook() is None:
        if "/root/.axon_site" not in sys.path:
            sys.path.insert(0, "/root/.axon_site")
        from trn_agent_boot.trn_boot import _ntff_profile_via_ctypes
        set_axon_ntff_profile_hook(
            _ntff_profile_via_ctypes("/opt/axon/libaxon_pjrt.so"))
    import concourse.bass_utils as bu
    bu.upload_artifacts = lambda tmpdir: tmpdir


def _run(inputs, trace=False, trace_kwargs=None):
    if trace:
        _install_trace_hooks()
    from concourse.bass_utils import run_bass_kernel_spmd
    nc = _build_program()
    in_maps = _host_prep(**inputs)
    res = run_bass_kernel_spmd(
        nc, in_maps, list(range(N_CORES)), trace=trace,
        **(trace_kwargs or {}))
    bv = inputs["bv"].astype(np.float32)
    bv = bv if np.abs(bv).max() > 0 else None
    out = _assemble(res.results, bv)
    return out, res


def kernel(x, Wq, bq, Wk, bk, Wv, bv):
    out, _ = _run(dict(x=np.asarray(x), Wq=np.asarray(Wq), bq=np.asarray(bq),
                       Wk=np.asarray(Wk), bk=np.asarray(bk),
                       Wv=np.asarray(Wv), bv=np.asarray(bv)))
    return out


# revision 18
# speedup vs baseline: 1.2039x; 1.0046x over previous
"""GQA kernel for Trainium2, 8 NeuronCores, tensor-parallel over heads.

Problem: B=1, T=2048, C=4096, 32 q-heads, 16 kv-heads, head_dim=128,
scale = 1/sqrt(32), causal. q head H uses kv head H%16.

Sharding (no collectives needed): core c owns q-heads
{2c, 2c+1, 2c+16, 2c+17} and kv-heads {2c, 2c+1}. Each output column
block depends only on its own head, so the full output is a host-side
concat of per-core column slices.

Per-core kernel (all matmuls bf16, fp32 PSUM accumulation):
  x resident in SBUF as [128, half*32K + kc*1024 + t'] (16 1MB chunk
  tiles; DMA order interleaves the first weight halves with the first
  x chunks so projection matmuls start as soon as chunk 0 lands).
  Startup: q0+k0+v0 projected kc-major with 6 psum accumulators
  (t4 0,1 then 2,3), tracking x-chunk arrival; later units t4-major.
  v strips are PE-transposed into vt ([tk,129] tiles with a ones
  column for the row-sum trick); 4 transposes share one psum bank
  (single-start trick) and drain with one strided DVE copy.
  Attention per head (4 Tq blocks of 512, Tk pairs of 2x128):
    S^T pair = kt^T @ qt -> [128,1024] PSUM, exp (ACT, scale folded),
    causal mask via {0,1} multiply on diagonal tiles (trimmed free
    dims on diagonal pairs), PV: pt slices as stationary, rhs v
    [tk,129]; out normalized by reciprocal(row-sum col).
  attn3 runs blocks 3..0 after q3's t4 3..0 so the kernel tail is the
  smallest block.
"""

import numpy as np
import ml_dtypes

BF16 = ml_dtypes.bfloat16
T = 2048
C = 4096
D = 128
N_HEADS = 32
N_KV_HEADS = 16
SCALE = float(1.0 / np.sqrt(np.float32(N_HEADS)))
KC = C // 128          # 32 contraction chunks
NQH = 4                # local q heads per core
NKV = 2                # local kv heads per core
NT = T // 128          # 16 token tiles
VROW = D + 1           # 129: v with ones column
N_CORES = 8
XCOLS = 2 * KC * 1024

_prog_cache = {}


def _build_program():
    if "nc" in _prog_cache:
        return _prog_cache["nc"]
    import concourse.bass as bass
    import concourse.tile as tile
    from concourse import bacc, mybir

    dt = mybir.dt
    f32 = dt.float32
    bf16 = dt.bfloat16
    EXP = mybir.ActivationFunctionType.Exp

    nc = bacc.Bacc("TRN2", target_bir_lowering=False, debug=False,
                   num_devices=N_CORES)

    xh_d = nc.dram_tensor("xh", [128, XCOLS], bf16, kind="ExternalInput").ap()
    wq_d = nc.dram_tensor("wq", [NQH, 128, C], bf16, kind="ExternalInput").ap()
    wk_d = nc.dram_tensor("wk", [NKV, 128, C], bf16, kind="ExternalInput").ap()
    wv_d = nc.dram_tensor("wv", [NKV, 128, C], bf16, kind="ExternalInput").ap()
    # masks: 4x [128,512] causal tiles + [128,128] identity for PE transpose
    mask_d = nc.dram_tensor("masks", [128, 4 * 512 + 128], bf16,
                            kind="ExternalInput").ap()
    out_d = nc.dram_tensor("out", [T, NQH * D], f32, kind="ExternalOutput").ap()

    with tile.TileContext(nc) as tc:
        with (
            tc.tile_pool(name="persist", bufs=1) as persist,
            tc.tile_pool(name="wpool", bufs=3) as wpool,
            tc.tile_pool(name="vtsp", bufs=1) as vtsp,
            tc.tile_pool(name="ptpool", bufs=3) as ptpool,
            tc.tile_pool(name="opool", bufs=4) as opool,
            tc.tile_pool(name="recpool", bufs=4) as recpool,
            tc.tile_pool(name="psum", bufs=4, space=bass.MemorySpace.PSUM) as psum,
            tc.tile_pool(name="psum2", bufs=2, space=bass.MemorySpace.PSUM) as psum2,
        ):
            mask_sb = persist.tile([128, 4 * 512 + 128], bf16, name="mask_sb",
                                   tag="mask_sb")
            ident = mask_sb[:, 4 * 512: 4 * 512 + 128]

            qt = persist.tile([128, NQH * T], bf16, name="qt", tag="qt")
            kt = persist.tile([128, NKV * T], bf16, name="kt", tag="kt")
            vt = persist.tile([128, NKV * NT * VROW], bf16, name="vt", tag="vt")

            # ones columns of v (row-sum trick)
            for i in range(NKV * NT):
                nc.vector.memset(vt[:, i * VROW + D: (i + 1) * VROW], 1.0)

            # x chunk layout: (half, kc_lo, n_kc); first two chunks smaller
            # so the first matmuls start as early as possible.
            xchunks = ([(0, 0, 2), (0, 2, 2)]
                       + [(0, k, 4) for k in range(4, KC, 4)]
                       + [(1, k, 4) for k in range(0, KC, 4)])
            xmap = {}
            xtiles = {}
            wts = {}

            def dma_x(ci):
                half, klo, nk = xchunks[ci]
                xc = persist.tile([128, nk * 1024], bf16, name=f"xc{ci}",
                                  tag=f"xc{ci}")
                base = half * (KC * 1024) + klo * 1024
                nc.sync.dma_start(out=xc[:],
                                  in_=xh_d[:, base:base + nk * 1024])
                xtiles[ci] = xc
                for k in range(klo, klo + nk):
                    xmap[(half, k)] = (ci, (k - klo) * 1024)

            def xs(t4, kc):
                ci, off = xmap[(t4 // 2, kc)]
                off += (t4 % 2) * 512
                return xtiles[ci][:, off:off + 512]

            def dma_w(src, idx, key, cols=None):
                if key not in wts:
                    w = wpool.tile([128, C], bf16, name=f"w_{key}", tag="w")
                    wts[key] = w
                w = wts[key]
                if cols is None:
                    nc.sync.dma_start(out=w[:], in_=src[idx])
                else:
                    lo, hi = cols
                    nc.sync.dma_start(out=w[:, lo:hi], in_=src[idx][:, lo:hi])

            def emit_tr_group(vts, kv, t4):
                """PE-transpose 4 [128,128] v tiles into one psum bank
                (single-start trick), one strided DVE copy into vt."""
                trp = psum.tile([128, 512], bf16, name=f"tr_{kv}_{t4}",
                                tag="ps")
                for rr in range(4):
                    j = t4 * 4 + rr
                    nc.tensor.matmul(
                        trp[:, rr * 128:(rr + 1) * 128],
                        lhsT=vts[:, j * 128:(j + 1) * 128],
                        rhs=ident, is_transpose=True,
                        start=(rr == 0), stop=(rr == 3),
                        skip_group_check=True)
                for rr in range(4):
                    j = t4 * 4 + rr
                    nc.vector.tensor_copy(
                        out=vt[:, (kv * NT + j) * VROW:
                               (kv * NT + j) * VROW + D],
                        in_=trp[:, rr * 128:(rr + 1) * 128])

            def startup():
                """q0+k0+v0 kc-major with 6 accumulators, per x half."""
                wq0, wk0, wv0 = wts["q0"], wts["k0"], wts["v0"]
                vts = vtsp.tile([128, T], bf16, name="vts_0", tag="vts")
                for half in (0, 1):
                    aq = psum2.tile([128, 1024], f32, name=f"aq{half}",
                                    tag="sp2")
                    accq = [aq[:, 0:512], aq[:, 512:1024]]
                    acck = [psum.tile([128, 512], f32, name=f"ak{half}{i}",
                                      tag="ps")[:] for i in (0, 1)]
                    accv = [psum.tile([128, 512], f32, name=f"av{half}{i}",
                                      tag="ps")[:] for i in (0, 1)]
                    for kc in range(KC):
                        for acc, w in ((accq, wq0), (acck, wk0), (accv, wv0)):
                            for i in (0, 1):
                                nc.tensor.matmul(
                                    acc[i],
                                    lhsT=w[:, kc * 128:(kc + 1) * 128],
                                    rhs=xs(2 * half + i, kc),
                                    start=(kc == 0), stop=(kc == KC - 1))
                    for i in (0, 1):
                        t4 = 2 * half + i
                        nc.vector.tensor_copy(
                            out=qt[:, t4 * 512:(t4 + 1) * 512], in_=accq[i])
                        nc.vector.tensor_copy(
                            out=kt[:, t4 * 512:(t4 + 1) * 512], in_=acck[i])
                        nc.vector.tensor_copy(
                            out=vts[:, t4 * 512:(t4 + 1) * 512], in_=accv[i])
                    emit_tr_group(vts, 0, 2 * half)
                    emit_tr_group(vts, 0, 2 * half + 1)

            def proj(wkey, dest, dbase, vts_kv=None, t4_order=(0, 1, 2, 3)):
                """t4-major projection of one [128, T] strip."""
                w = wts[wkey]
                vts = None
                if vts_kv is not None:
                    vts = vtsp.tile([128, T], bf16, name=f"vts_{wkey}",
                                    tag="vts")
                with nc.named_scope(f"proj_{wkey}"):
                    for t4 in t4_order:
                        ps = psum.tile([128, 512], f32,
                                       name=f"ps_{wkey}_{t4}", tag="ps")
                        for kc in range(KC):
                            nc.tensor.matmul(
                                ps[:], lhsT=w[:, kc * 128:(kc + 1) * 128],
                                rhs=xs(t4, kc),
                                start=(kc == 0), stop=(kc == KC - 1))
                        if vts is None:
                            nc.vector.tensor_copy(
                                out=dest[:, dbase + t4 * 512:
                                         dbase + (t4 + 1) * 512],
                                in_=ps[:])
                        else:
                            nc.vector.tensor_copy(
                                out=vts[:, t4 * 512:(t4 + 1) * 512], in_=ps[:])
                    if vts is not None:
                        for t4 in range(4):
                            emit_tr_group(vts, vts_kv, t4)

            def attn(h, post_b0=None):
                kv = h % 2

                def drain(h, b, s, pv):
                    rec = recpool.tile([128, 1], f32,
                                       name=f"rec_{h}_{b}_{s}", tag="rec")
                    nc.vector.reciprocal(rec[:], pv[:, D:D + 1])
                    ot = opool.tile([128, 128], f32,
                                    name=f"ot_{h}_{b}_{s}", tag="ot")
                    nc.vector.tensor_scalar_mul(ot[:], pv[:, 0:D], rec[:])
                    nc.sync.dma_start(
                        out=out_d[b * 512 + s * 128:b * 512 + (s + 1) * 128,
                                  h * D:(h + 1) * D],
                        in_=ot[:])

                with nc.named_scope(f"attn_{h}"):
                    for b in (0, 1, 2, 3):
                        pvs = []
                        for s in range(4):
                            pv = psum.tile([128, 512], f32,
                                           name=f"pv_{h}_{b}_{s}", tag="ps")
                            pvs.append(pv)
                        for p in range(2 * b + 2):  # pairs of Tk tiles
                            spp = psum2.tile([128, 1024], f32,
                                             name=f"sp_{h}_{b}_{p}", tag="sp2")
                            pt = ptpool.tile([128, 1024], bf16,
                                             name=f"pt_{h}_{b}_{p}", tag="pt")
                            qsl = qt[:, h * T + b * 512:h * T + (b + 1) * 512]
                            for half in range(2):
                                j = 2 * p + half
                                nc.tensor.matmul(
                                    spp[:, half * 512:(half + 1) * 512],
                                    lhsT=kt[:, kv * T + j * 128:
                                            kv * T + (j + 1) * 128],
                                    rhs=qsl,
                                    start=True, stop=True,
                                )
                            nc.scalar.activation(pt[:], spp[:], EXP,
                                                 scale=SCALE)
                            if p >= 2 * b:  # diagonal pairs, mask-strip aligned
                                roff = (p - 2 * b) * 1024
                                nc.vector.tensor_mul(
                                    pt[:], pt[:],
                                    mask_sb[:, roff:roff + 1024])
                            for half in range(2):
                                j = 2 * p + half
                                r = j - 4 * b
                                vsl = vt[:, (kv * NT + j) * VROW:
                                         (kv * NT + j + 1) * VROW]
                                for s in range(max(0, r), 4):
                                    nc.tensor.matmul(
                                        pvs[s][:, 0:VROW],
                                        lhsT=pt[:, half * 512 + s * 128:
                                                half * 512 + (s + 1) * 128],
                                        rhs=vsl,
                                        start=(j == 0), stop=(j == 4 * b + s),
                                    )
                            # drain accumulators whose last term just landed
                            if p == 2 * b:
                                drain(h, b, 0, pvs[0])
                                drain(h, b, 1, pvs[1])
                            elif p == 2 * b + 1:
                                drain(h, b, 2, pvs[2])
                                drain(h, b, 3, pvs[3])
                        if b == 0 and post_b0 is not None:
                            post_b0()

            # ---- DMA schedule (ring is FIFO in issue order) ----
            # Weight eighths (4 kc each) for the three startup units
            # interleaved with the x chunks that consume them; startup
            # compute is PE-bound from the first chunk.
            E = C // 8
            # h0 x chunk index covering kc group g: chunks 0,1 are kc0-1,2-3
            h0_chunk_for_g = {0: (0, 1), 1: (2,), 2: (3,), 3: (4,),
                              4: (5,), 5: (6,), 6: (7,), 7: (8,)}
            xi_done = set()
            for g in range(8):
                for key, src in (("q0", wq_d), ("k0", wk_d), ("v0", wv_d)):
                    dma_w(src, 0, key, cols=(g * E, (g + 1) * E))
                for ci in h0_chunk_for_g[g]:
                    if ci not in xi_done:
                        dma_x(ci)
                        xi_done.add(ci)
                if g == 3:
                    nc.sync.dma_start(out=mask_sb[:], in_=mask_d[:])
            for ci in range(9, 17):
                dma_x(ci)

            # ---- compute schedule ----
            startup()

            def _later_w():
                dma_w(wq_d, 2, "q2")
                dma_w(wk_d, 1, "k1")
                dma_w(wv_d, 1, "v1")

            attn(0, post_b0=_later_w)
            proj("q2", qt, 2 * T)
            dma_w(wq_d, 1, "q1")
            attn(2)
            proj("k1", kt, T)
            dma_w(wq_d, 3, "q3")
            proj("v1", None, 0, vts_kv=1)
            proj("q1", qt, T)
            attn(1)
            proj("q3", qt, 3 * T)
            attn(3)

    nc.compile()
    _prog_cache["nc"] = nc
    return nc


def _host_prep(x, Wq, bq, Wk, bk, Wv, bv):
    """Shard + repack inputs for the 8 cores. Returns in_maps list."""
    assert x.shape == (1, T, C)
    assert np.abs(bq).max() == 0 and np.abs(bk).max() == 0, \
        "nonzero q/k biases not supported"

    x0 = np.ascontiguousarray(x[0]).astype(BF16)
    # xh packed: [128, half*32K + kc*1024 + t'] = x[half*1024+t', kc*128+p]
    xh = np.ascontiguousarray(
        x0.reshape(2, 1024, KC, 128).transpose(3, 0, 2, 1).reshape(128, XCOLS))

    # causal masks for the 4 diagonal-tile offsets: mask_r[tk,tq] = tq >= tk+128r
    tq = np.arange(512)[None, :]
    tk = np.arange(128)[:, None]
    masks = np.concatenate(
        [(tq >= (tk + 128 * r)).astype(BF16) for r in range(4)]
        + [np.eye(128, dtype=BF16)], axis=1)
    masks = np.ascontiguousarray(masks)

    def pack_w(Wrows):
        # Wrows: [128 (out c), C (in)] for one head ->
        # packed[p, 128*kc + c] = Wrows[c, 128*kc + p]
        return np.ascontiguousarray(
            Wrows.astype(BF16).reshape(128, KC, 128).transpose(2, 1, 0)
            .reshape(128, C))

    in_maps = []
    for c in range(N_CORES):
        qheads = [2 * c, 2 * c + 1, 2 * c + 16, 2 * c + 17]
        kvheads = [2 * c, 2 * c + 1]
        wq = np.stack([pack_w(Wq[128 * H:128 * (H + 1)]) for H in qheads])
        wk = np.stack([pack_w(Wk[128 * K:128 * (K + 1)]) for K in kvheads])
        wv = np.stack([pack_w(Wv[128 * K:128 * (K + 1)]) for K in kvheads])
        in_maps.append({
            "xh": xh, "wq": wq, "wk": wk, "wv": wv, "masks": masks,
        })
    return in_maps


def _assemble(results, bv):
    out = np.empty((T, C), dtype=np.float32)
    for c in range(N_CORES):
        r = results[c]["out"]
        qheads = [2 * c, 2 * c + 1, 2 * c + 16, 2 * c + 17]
        for i, H in enumerate(qheads):
            blk = r[:, 128 * i:128 * (i + 1)]
            if bv is not None:
                blk = blk + bv[128 * (H % N_KV_HEADS):
                               128 * (H % N_KV_HEADS) + 128]
            out[:, 128 * H:128 * (H + 1)] = blk
    return out.reshape(1, T, C)


def _install_trace_hooks():
    """The agent image's antenv lacks axon_hooks; recreate it so
    run_bass_kernel_spmd's trace=True path can capture NTFF profiles."""
    import sys
    import types
    import antenv
    if "antenv.axon_hooks" not in sys.modules:
        mod = types.ModuleType("antenv.axon_hooks")
        mod._hook = None

        def set_axon_ntff_profile_hook(h):
            mod._hook = h

        def get_axon_ntff_profile_hook():
            return mod._hook

        mod.set_axon_ntff_profile_hook = set_axon_ntff_profile_hook
        mod.get_axon_ntff_profile_hook = get_axon_ntff_profile_hook
        sys.modules["antenv.axon_hooks"] = mod
        antenv.axon_hooks = mod
    from antenv.axon_hooks import (get_axon_ntff_profile_hook,
                                   set_axon_ntff_profile_hook)
    if get_axon_ntff_profile_hook() is None:
        if "/root/.axon_site" not in sys.path:
            sys.path.insert(0, "/root/.axon_site")
        from trn_agent_boot.trn_boot import _ntff_profile_via_ctypes
        set_axon_ntff_profile_hook(
            _ntff_profile_via_ctypes("/opt/axon/libaxon_pjrt.so"))
    import concourse.bass_utils as bu
    bu.upload_artifacts = lambda tmpdir: tmpdir


def _run(inputs, trace=False, trace_kwargs=None):
    if trace:
        _install_trace_hooks()
    from concourse.bass_utils import run_bass_kernel_spmd
    nc = _build_program()
    in_maps = _host_prep(**inputs)
    res = run_bass_kernel_spmd(
        nc, in_maps, list(range(N_CORES)), trace=trace,
        **(trace_kwargs or {}))
    bv = inputs["bv"].astype(np.float32)
    bv = bv if np.abs(bv).max() > 0 else None
    out = _assemble(res.results, bv)
    return out, res


def kernel(x, Wq, bq, Wk, bk, Wv, bv):
    out, _ = _run(dict(x=np.asarray(x), Wq=np.asarray(Wq), bq=np.asarray(bq),
                       Wk=np.asarray(Wk), bk=np.asarray(bk),
                       Wv=np.asarray(Wv), bv=np.asarray(bv)))
    return out
